# revision 1
# baseline (speedup 1.0000x reference)
"""Trainium2 Bass kernel for nn_BinaryResNetBlock (XNOR-style binary ResNet block).

Math (per reference):
  a1 = sign(x);  y1 = conv3x3(a1, s1*sign(w1));  inner = sign(BN_train(y1))
  y2 = conv3x3(inner, s2*sign(w2));  out = sign(BN_train(y2) + x)

Key facts exploited:
  - conv inputs are exactly {-1,0,+1} and sign(w) in {-1,0,+1}: the conv result
    is (per-channel scale s) * (exact small integer n). We compute n exactly with
    fp8 DoubleRow matmuls accumulating in fp32 PSUM (|n| <= 2304 < 2^24).
  - n is exactly representable in fp16, so conv integers are stored fp16.
  - BN(y)*gamma+beta = A*n + B with per-channel A, B derived from global batch
    stats => cross-core AllReduce of tiny per-channel sums only.
  - sign(BN1(y1)) with beta1==0 reduces to sign(gamma1*(n - mean_n)): no var
    needed for stage 1 in the fast path.

Sharding: data-parallel over batch, 8 images per core on 8 NeuronCores;
weights/BN params replicated; tiny AllReduces for the BN stats.

Scheduling: convs run o-chunk-major; each o-chunk's stats reduce + AllReduce +
affine (and, for conv2, the final elementwise pass) overlap the other o-chunk's
matmuls. a_buf is reused in place for a1 -> inner, m_buf for m1 -> m2 (Tile
subtile deps order the overwrites).
"""

import numpy as np
import ml_dtypes

# ---- problem constants (hardcoded; kernel.py must be self-contained) ----
NCORES = 8
NTOT = 64          # total batch
NIMG = NTOT // NCORES
CH = 256           # in/out channels
H = W = 56
HW = H * W         # 3136
WPAD = 58          # padded width (one col pad each side)
IMG_STR = 3366     # padded image stride: 58*58 (+2 tail pad, even)
RH = 8             # output rows per tile
RG = H // RH       # 7 row groups
FREE = RH * WPAD   # 464 matmul free size (includes junk columns)
VW = RH * W        # 448 valid elements per tile
NTILE = NIMG * RG  # 56 tiles per (oc)
COUNT_TOT = NTOT * HW  # 200704 elements per channel for BN stats
EPS = 1e-5

_CACHE = {}
LAST_RESULT = None  # BassKernelResults of the most recent run (for test harness)


def _build(fast1: bool, dbg: bool = False, reps: int = 1, sync_level: int = 1):
    import concourse.bacc as bacc
    import concourse.mybir as mybir
    import concourse.tile as tile
    from itertools import product

    F8 = mybir.dt.float8e4
    F16 = mybir.dt.float16
    F32 = mybir.dt.float32
    AF = mybir.ActivationFunctionType
    ALU = mybir.AluOpType
    DR = mybir.MatmulPerfMode.DoubleRow

    nc = bacc.Bacc("TRN2", target_bir_lowering=False, debug=False,
                   enable_asserts=True, num_devices=NCORES)

    x_d = nc.dram_tensor("x", [NIMG, CH, H, W], F32, kind="ExternalInput").ap()
    wq_d = nc.dram_tensor("wq", [128, 2, 3, 3, 2, 2, 128], F8,
                          kind="ExternalInput").ap()
    cf_d = nc.dram_tensor("cf", [128, 2, 6], F32, kind="ExternalInput").ap()
    out_d = nc.dram_tensor("out", [NIMG, CH, H, W], F32,
                           kind="ExternalOutput").ap()
    if dbg:
        da1_d = nc.dram_tensor("d_a1", [128, 2, NIMG, IMG_STR], F8,
                               kind="ExternalOutput").ap()
        da2_d = nc.dram_tensor("d_a2", [128, 2, NIMG, IMG_STR], F8,
                               kind="ExternalOutput").ap()
        dm1_d = nc.dram_tensor("d_m1", [128, 2, NIMG, HW], F16,
                               kind="ExternalOutput").ap()
        dm2_d = nc.dram_tensor("d_m2", [128, 2, NIMG, HW], F16,
                               kind="ExternalOutput").ap()
        dab1_d = nc.dram_tensor("d_ab1", [128, 2, 2], F32,
                                kind="ExternalOutput").ap()
        dab2_d = nc.dram_tensor("d_ab2", [128, 2, 2], F32,
                                kind="ExternalOutput").ap()

    with tile.TileContext(nc) as tc:
        with tc.tile_pool(name="big", bufs=1) as big, \
             tc.tile_pool(name="small", bufs=1) as small, \
             tc.tile_pool(name="xst", bufs=3) as xst, \
             tc.tile_pool(name="fin", bufs=6) as fin, \
             tc.tile_pool(name="ps", bufs=8, space="PSUM") as pspool, \
             tc.tile_pool(name="dr", bufs=1, space="DRAM") as dr:

            # persistent buffers
            a_buf = big.tile([128, 2, NIMG, IMG_STR], F8)    # padded +-1 acts
            m_buf = big.tile([128, 2, NIMG, HW], F16)        # conv ints (m1 then m2)
            wsb = small.tile([128, 2, 3, 3, 2, 2, 128], F8)  # signed weights
            cf = small.tile([128, 2, 6], F32)                # s1,g1,b1,s2,g2,b2
            acc1 = small.tile([128, 2, NTILE], F32)          # per-tile sums (conv1)
            st1 = small.tile([128, 2, NTILE, 6], F32)        # bn_stats (conv1, general)
            st2 = small.tile([128, 2, NTILE, 6], F32)        # bn_stats (conv2)
            ab1 = small.tile([128, 2, 2], F32)               # A1, B1
            ab2 = small.tile([128, 2, 2], F32)               # A2, B2

            nc.sync.dma_start(wsb[:], wq_d[:])
            nc.sync.dma_start(cf[:], cf_d[:])

            # zero the padding cells of a_buf (stay zero for both conv inputs);
            # 3 strided memsets covering all (j, img) at once
            nc.gpsimd.memset(a_buf[:, :, :, 0:WPAD], 0.0)              # row -1
            nc.gpsimd.memset(a_buf[:, :, :, 57 * WPAD:IMG_STR], 0.0)   # row 56 + tail
            colpad = a_buf[:, :, :, 57:57 + 57 * WPAD].rearrange(
                "p j i (r t) -> p j i r t", t=WPAD)[:, :, :, :, 0:2]   # col pads
            nc.gpsimd.memset(colpad, 0.0)

            def interior(j, img, r0, nrows):
                """[128, nrows, 56] view of the valid cells of a_buf."""
                return a_buf[:, j, img, 0:3364].rearrange(
                    "p (r w) -> p r w", w=WPAD)[:, 1 + r0:1 + r0 + nrows, 1:57]

            # per-channel affine from globally-reduced [mean,var,mean^2] sums:
            # A = gamma*s*r, B = beta - mean_n*A, r = 1/sqrt(s^2*var_n+eps)
            def make_affine(gsb, ab, oc, si, gi, bi):
                gm = small.tile([128, 1], F32, name=f"gm{si}_{oc}")
                gv = small.tile([128, 1], F32, name=f"gv{si}_{oc}")
                gq = small.tile([128, 1], F32, name=f"gq{si}_{oc}")
                t0 = small.tile([128, 1], F32, name=f"t0{si}_{oc}")
                t1 = small.tile([128, 1], F32, name=f"t1{si}_{oc}")
                sc = cf[:, oc, si:si + 1]
                gc = cf[:, oc, gi:gi + 1]
                bc = cf[:, oc, bi:bi + 1]
                nc.vector.tensor_scalar_mul(gm[:], gsb[:, 0:1], 1.0 / NCORES)
                nc.vector.tensor_scalar_mul(gv[:], gsb[:, 1:2], 1.0 / NCORES)
                nc.vector.tensor_scalar_mul(gq[:], gsb[:, 2:3], 1.0 / NCORES)
                # var_n = gv + gq - gm^2
                nc.vector.tensor_tensor(t0[:], gm[:], gm[:], ALU.mult)
                nc.vector.tensor_sub(t0[:], gq[:], t0[:])
                nc.vector.tensor_add(t0[:], gv[:], t0[:])
                # t0 = s^2*var_n + eps
                nc.vector.tensor_tensor(t1[:], sc, sc, ALU.mult)
                nc.vector.tensor_tensor(t0[:], t0[:], t1[:], ALU.mult)
                nc.vector.tensor_scalar_add(t0[:], t0[:], EPS)
                # r = 1/sqrt(t0)
                nc.scalar.sqrt(t0[:], t0[:])
                nc.vector.reciprocal(t0[:], t0[:])
                # A = gamma*s*r
                nc.vector.tensor_tensor(t1[:], gc, sc, ALU.mult)
                nc.vector.tensor_tensor(t1[:], t1[:], t0[:], ALU.mult)
                nc.vector.tensor_copy(ab[:, oc, 0:1], t1[:])
                # B = beta - mean_n*A
                nc.vector.tensor_tensor(t1[:], gm[:], t1[:], ALU.mult)
                nc.vector.tensor_sub(ab[:, oc, 1:2], bc, t1[:])

            def allreduce(payload, width, tag):
                ci = dr.tile([128, width], F32, name=f"ci_{tag}")
                co = dr.tile([128, width], F32, name=f"co_{tag}")
                nc.sync.dma_start(ci[:], payload)
                nc.gpsimd.collective_compute(
                    "AllReduce", ALU.add,
                    replica_groups=[list(range(NCORES))],
                    ins=[ci.opt()], outs=[co.opt()])
                g = small.tile([128, width], F32, name=f"g_{tag}")
                nc.sync.dma_start(g[:], co[:])
                return g

            def body(rep):
                # ---- stage 0: a1 = sign(x), fp32 HBM -> fp8 padded SBUF ----
                for img in range(NIMG):
                    for j in range(2):
                        for hf in range(2):
                            xt = xst.tile([128, HW // 2], F32, tag="xst")
                            nc.sync.dma_start(
                                xt[:],
                                x_d[img].rearrange("c h w -> c (h w)")
                                [j * 128:(j + 1) * 128,
                                 hf * (HW // 2):(hf + 1) * (HW // 2)])
                            nc.scalar.activation(
                                interior(j, img, hf * 28, 28),
                                xt.rearrange("p (h w) -> p h w", w=W), AF.Sign)

                # ---- one o-chunk of a conv ----
                def conv_oc(cv, oc):
                    for img in range(NIMG):
                        for rg in range(RG):
                            pt = pspool.tile([128, FREE], F32, tag="ps")
                            for t, (dh, dw) in enumerate(
                                    product(range(3), range(3))):
                                s = (rg * RH + dh) * WPAD + dw
                                rhs = a_buf[:, :, img, s:s + FREE]
                                nc.tensor.matmul(
                                    pt[:], wsb[:, cv, dh, dw, oc], rhs,
                                    start=(t == 0), stop=(t == 8), perf_mode=DR)
                            pv = pt.rearrange(
                                "p (r w) -> p r w", w=WPAD)[:, :, 0:56]
                            mflat = m_buf[:, oc, img, rg * VW:(rg + 1) * VW]
                            mv = mflat.rearrange("p (r w) -> p r w", w=W)
                            ti = img * RG + rg
                            if cv == 0:
                                if fast1:
                                    nc.vector.tensor_scalar(
                                        mv, pv, 1.0, 0.0, ALU.mult, ALU.add,
                                        accum_out=acc1[:, oc, ti:ti + 1])
                                else:
                                    if ti % 2 == 0:
                                        nc.scalar.copy(mv, pv)
                                    else:
                                        nc.vector.tensor_scalar(
                                            mv, pv, 1.0, 0.0, ALU.mult, ALU.add)
                                    nc.vector.bn_stats(st1[:, oc, ti, :], mflat)
                            else:
                                # 2/3 of evictions on ACT: DVE also carries
                                # bn_stats, ACT also carries the sign-pass
                                if ti % 3 != 2:
                                    nc.scalar.copy(mv, pv)
                                else:
                                    nc.vector.tensor_scalar(
                                        mv, pv, 1.0, 0.0, ALU.mult, ALU.add)
                                nc.vector.bn_stats(st2[:, oc, ti, :], mflat)

                # ---- conv1 per-oc stats -> ab1 ----
                def stats1_oc(oc):
                    if fast1:
                        sum1 = small.tile([128, 1], F32, name=f"sum1_{oc}")
                        nc.vector.tensor_reduce(
                            sum1[:], acc1[:, oc, :], mybir.AxisListType.X,
                            ALU.add)
                        g1 = allreduce(sum1[:], 1, f"s1_{oc}")
                        # A1 = gamma1 ; B1 = -gamma1 * mean_n
                        mu1 = small.tile([128, 1], F32, name=f"mu1_{oc}")
                        nc.vector.tensor_scalar_mul(mu1[:], g1[:],
                                                    -1.0 / COUNT_TOT)
                        nc.vector.tensor_copy(ab1[:, oc, 0:1],
                                              cf[:, oc, 1:2])
                        nc.vector.tensor_tensor(ab1[:, oc, 1:2], mu1[:],
                                                cf[:, oc, 1:2], ALU.mult)
                    else:
                        ag1 = small.tile([128, 2], F32, name=f"ag1_{oc}")
                        pl1 = small.tile([128, 3], F32, name=f"pl1_{oc}")
                        nc.vector.bn_aggr(ag1[:], st1[:, oc].rearrange(
                            "p t (u s) -> p (t u) s", s=3))
                        nc.vector.tensor_copy(pl1[:, 0:2], ag1[:])
                        nc.vector.tensor_tensor(pl1[:, 2:3], ag1[:, 0:1],
                                                ag1[:, 0:1], ALU.mult)
                        g1 = allreduce(pl1[:], 3, f"s1_{oc}")
                        make_affine(g1, ab1, oc, 0, 1, 2)

                def stats2_oc(oc):
                    ag2 = small.tile([128, 2], F32, name=f"ag2_{oc}")
                    pl2 = small.tile([128, 3], F32, name=f"pl2_{oc}")
                    nc.vector.bn_aggr(ag2[:], st2[:, oc].rearrange(
                        "p t (u s) -> p (t u) s", s=3))
                    nc.vector.tensor_copy(pl2[:, 0:2], ag2[:])
                    nc.vector.tensor_tensor(pl2[:, 2:3], ag2[:, 0:1],
                                            ag2[:, 0:1], ALU.mult)
                    g2 = allreduce(pl2[:], 3, f"s2_{oc}")
                    make_affine(g2, ab2, oc, 3, 4, 5)

                # ---- final: out = sign(A2*m2 + B2 + x) for one oc ----
                def final_oc(oc):
                    for img in range(NIMG):
                        for rg in range(RG):
                            mflat = m_buf[:, oc, img, rg * VW:(rg + 1) * VW]
                            tt = fin.tile([128, VW], F32, tag="tt")
                            nc.vector.tensor_scalar(
                                tt[:], mflat, ab2[:, oc, 0:1], ab2[:, oc, 1:2],
                                ALU.mult, ALU.add)
                            xs = x_d[img].rearrange("c h w -> c (h w)")[
                                oc * 128:(oc + 1) * 128,
                                rg * VW:(rg + 1) * VW]
                            nc.gpsimd.dma_start(tt[:], xs, accum_op=ALU.add)
                            ot = fin.tile([128, VW], F32, tag="ot")
                            nc.scalar.activation(ot[:], tt[:], AF.Sign)
                            nc.sync.dma_start(
                                out_d[img].rearrange("c h w -> c (h w)")[
                                    oc * 128:(oc + 1) * 128,
                                    rg * VW:(rg + 1) * VW],
                                ot[:])

                # ---- conv1: oc0 stats/allreduce overlap oc1 matmuls ----
                conv_oc(0, 0)
                stats1_oc(0)
                conv_oc(0, 1)
                if sync_level >= 1:
                    tc.strict_bb_all_engine_barrier()
                stats1_oc(1)

                if dbg:
                    nc.sync.dma_start(da1_d[:], a_buf[:])
                    nc.sync.dma_start(dm1_d[:], m_buf[:])
                    nc.sync.dma_start(dab1_d[:], ab1[:])
                    tc.strict_bb_all_engine_barrier()

                # ---- inner = sign(A1*n + B1), overwrites a_buf in place ----
                for img in range(NIMG):
                    for oc in range(2):
                        src = m_buf[:, oc, img].rearrange(
                            "p (h w) -> p h w", w=W)
                        nc.scalar.activation(
                            interior(oc, img, 0, 56), src, AF.Sign,
                            bias=ab1[:, oc, 1:2], scale=ab1[:, oc, 0:1])

                if dbg:
                    nc.sync.dma_start(da2_d[:], a_buf[:])
                    tc.strict_bb_all_engine_barrier()

                # ---- conv2: oc0 stats+final overlap oc1 matmuls ----
                conv_oc(1, 0)
                stats2_oc(0)
                conv_oc(1, 1)
                final_oc(0)
                if sync_level >= 1:
                    tc.strict_bb_all_engine_barrier()
                stats2_oc(1)
                final_oc(1)

                if dbg:
                    tc.strict_bb_all_engine_barrier()
                    nc.sync.dma_start(dm2_d[:], m_buf[:])
                    nc.sync.dma_start(dab2_d[:], ab2[:])

            for _rep in range(reps):
                body(_rep)

    nc.compile()
    return nc


def _prep_weights(w, F8NP):
    """[O,C,3,3] fp32 -> ([128k, 3, 3, 2oc, 2j, 128m] fp8 sign, [256] fp32 scale)."""
    scale = np.mean(np.abs(w), axis=(1, 2, 3), dtype=np.float32)
    ws = np.sign(w).astype(F8NP)
    arr = ws.reshape(2, 128, 2, 128, 3, 3)       # [oc, m, j, k, dh, dw]
    arr = arr.transpose(3, 4, 5, 0, 2, 1)        # [k, dh, dw, oc, j, m]
    return np.ascontiguousarray(arr), scale


def kernel(**inputs) -> np.ndarray:
    global LAST_RESULT
    import os
    from concourse import bass_utils

    x = np.ascontiguousarray(np.asarray(inputs["inputs"], dtype=np.float32))
    w1 = np.asarray(inputs["w1"], dtype=np.float32)
    w2 = np.asarray(inputs["w2"], dtype=np.float32)
    g1 = np.asarray(inputs["gamma1"], dtype=np.float32)
    b1 = np.asarray(inputs["beta1"], dtype=np.float32)
    g2 = np.asarray(inputs["gamma2"], dtype=np.float32)
    b2 = np.asarray(inputs["beta2"], dtype=np.float32)

    F8NP = ml_dtypes.float8_e4m3
    wq1, s1 = _prep_weights(w1, F8NP)
    wq2, s2 = _prep_weights(w2, F8NP)
    wq = np.ascontiguousarray(np.stack([wq1, wq2], axis=1))  # [128,2,3,3,2,2,128]

    coef = np.stack([s1, g1, b1, s2, g2, b2], axis=1)  # [256, 6]
    coef = np.ascontiguousarray(
        coef.reshape(2, 128, 6).transpose(1, 0, 2).astype(np.float32))

    fast1 = bool(np.all(b1 == 0.0))
    dbg = os.environ.get("KERNEL_DEBUG", "0") == "1"
    sync_level = int(os.environ.get("KERNEL_SYNC_LEVEL", "0"))

    key = (fast1, dbg, sync_level)
    if key not in _CACHE:
        _CACHE[key] = _build(fast1, dbg, sync_level=sync_level)
    nc = _CACHE[key]

    in_maps = [
        {"x": np.ascontiguousarray(x[i * NIMG:(i + 1) * NIMG]),
         "wq": wq, "cf": coef}
        for i in range(NCORES)
    ]
    res = bass_utils.run_bass_kernel_spmd(
        nc, in_maps, core_ids=list(range(NCORES)))
    LAST_RESULT = res
    out = np.concatenate([res.results[i]["out"] for i in range(NCORES)], axis=0)
    return out

